# revision 12
# baseline (speedup 1.0000x reference)
"""Trainium2 Bass kernel for nn_Node2Point (ragged gather/compact/resample).

Strategy: data-parallel over the 128 proposals across 8 cores (16 each).
Every core runs the identical cheap index computation for all 128 proposals
with partition-dim = proposal; per-core 0/1 matmuls extract that core's
16-proposal slice into the wrapped index-tile layout that dma_gather wants;
each core then performs only its own heavy feature-row gather and writes its
output shard.
"""

import sys

if "/opt/trn_rl_repo" not in sys.path:
    sys.path.insert(0, "/opt/trn_rl_repo")

import numpy as np

# dims (hardcoded per problem spec)
P, NB, K, NNODES, NPTS, D = 128, 16, 64, 512, 20000, 256
M = 450                 # FINEMATCH_MAX_POINT
L = NB * K              # 1024
NCORES = 8
PLOC = P // NCORES      # 16 proposals per core
NSLOT = PLOC * M        # 7200 gather slots per core
NIDX = 7296             # rounded up to multiple of 128 (456 idx cols * 16)
COLS = NIDX // 128      # 57
IDXC = NIDX // 16       # 456
NPAIR = NNODES * K // 2  # 16384 point pairs

_CACHE = {}


def _build_nc(stage=99):
    import concourse.mybir as mybir
    from concourse import bacc, tile

    AL = mybir.AluOpType
    f32, i16, u16, i32, u8 = (
        mybir.dt.float32,
        mybir.dt.int16,
        mybir.dt.uint16,
        mybir.dt.int32,
        mybir.dt.uint8,
    )

    nc = bacc.Bacc("TRN2", debug=False, num_devices=NCORES)

    # inputs
    seedw_d = nc.dram_tensor("seedw", [128, 128], i16, kind="ExternalInput")
    maskt_d = nc.dram_tensor("mask_tbl", [NNODES + 1, K], f32, kind="ExternalInput")
    cidxt_d = nc.dram_tensor("cidx_tbl", [NNODES + 1, K], f32, kind="ExternalInput")
    gpost_d = nc.dram_tensor("gpos2_tbl", [NNODES + 1, K], f32, kind="ExternalInput")
    pts_d = nc.dram_tensor("ptspair", [NPAIR + 1, 64], f32, kind="ExternalInput")
    feats_d = nc.dram_tensor("feats_pad", [NPTS + 1, D], f32, kind="ExternalInput")
    frac_d = nc.dram_tensor("frac_bc", [128, M], f32, kind="ExternalInput")
    iota_d = nc.dram_tensor("iota_bc", [128, M], f32, kind="ExternalInput")
    data1_d = nc.dram_tensor("data1", [128, M], i16, kind="ExternalInput")
    w_d = nc.dram_tensor("W", [128, 128], f32, kind="ExternalInput")
    w16_d = nc.dram_tensor("W16", [128, 16], f32, kind="ExternalInput")

    # outputs (per-core shard)
    featso_d = nc.dram_tensor("feats_out", [PLOC, M, D], f32, kind="ExternalOutput")
    ptso_d = nc.dram_tensor("pts_out", [PLOC, M, 3], f32, kind="ExternalOutput")
    masko_d = nc.dram_tensor("mask_out", [PLOC, M], u8, kind="ExternalOutput")

    with tile.TileContext(nc) as tc:
        with (
            tc.tile_pool(name="main", bufs=1) as pool,
            tc.tile_pool(name="ps", bufs=1, space="PSUM") as psum,
        ):
            t = lambda shape, dt, tag: pool.tile(shape, dt, tag=tag, name=tag)

            def body():
                # ---- load small inputs ----
                seedw = t([128, 128], i16, "seedw")
                nc.sync.dma_start(seedw[:], seedw_d.ap())
                if stage <= 0:
                    dbg0 = t([16, M], u8, "dbg0")
                    nc.vector.tensor_copy(dbg0[:], seedw[0:16, 0:128].broadcast_to([16, M]) if False else seedw[0:16, 0:1].broadcast_to([16, M]))
                    nc.sync.dma_start(masko_d.ap(), dbg0[:])
                    return
                frac = t([128, M], f32, "frac")
                nc.sync.dma_start(frac[:], frac_d.ap())
                iota = t([128, M], f32, "iota")
                nc.sync.dma_start(iota[:], iota_d.ap())
                data1 = t([128, M], i16, "data1")
                nc.sync.dma_start(data1[:], data1_d.ap())
                wt = t([128, 128], f32, "wt")
                nc.sync.dma_start(wt[:], w_d.ap())
                w16t = t([128, 16], f32, "w16t")
                nc.sync.dma_start(w16t[:], w16_d.ap())

                # ---- phase 1: table gathers -> [128 proposals, 1024] ----
                g_mask = t([128, L], f32, "g_mask")
                g_cidx = t([128, L], f32, "g_cidx")
                g_gpos = t([128, L], f32, "g_gpos")
                for tl, tbl in (
                    (g_mask, maskt_d),
                    (g_cidx, cidxt_d),
                    (g_gpos, gpost_d),
                ):
                    nc.gpsimd.dma_gather(
                        tl[:].rearrange("p (c k) -> p c k", k=K),
                        tbl.ap(),
                        seedw[:],
                        P * NB,
                        P * NB,
                        K,
                        single_packet=False,
                    )

                if stage <= 1:
                    dbg = t([16, M], u8, "dbg")
                    nc.vector.tensor_copy(dbg[:], g_mask[0:16, 0:M])
                    nc.sync.dma_start(masko_d.ap(), dbg[:])
                    return

                # ---- phase 2: scan / ranks / resample ----
                zeros = t([128, L], f32, "zeros")
                nc.vector.memset(zeros[:], 0.0)
                incl = t([128, L], f32, "incl")
                nc.vector.tensor_tensor_scan(
                    incl[:], g_mask[:], zeros[:], 0.0, AL.add, AL.add
                )
                n_t = t([128, 1], f32, "n_t")
                nc.vector.tensor_reduce(
                    n_t[:], g_mask[:], axis=mybir.AxisListType.X, op=AL.add
                )

                sidx_f = t([128, L], f32, "sidx_f")
                nc.vector.tensor_mul(sidx_f[:], incl[:], g_mask[:])
                nc.vector.tensor_scalar_add(sidx_f[:], sidx_f[:], -1.0)
                sidx = t([128, L], i16, "sidx")
                nc.vector.tensor_copy(sidx[:], sidx_f[:])

                cidx16 = t([128, L], i16, "cidx16")
                nc.vector.tensor_copy(cidx16[:], g_cidx[:])
                gpos16 = t([128, L], u16, "gpos16")
                nc.vector.tensor_copy(gpos16[:], g_gpos[:])

                # local[i] = max(floor(frac*n), i)
                t0 = t([128, M], f32, "t0")
                nc.vector.tensor_scalar(
                    t0[:], frac[:], n_t[:, 0:1], None, op0=AL.mult
                )
                ti32 = t([128, M], i32, "ti32")
                nc.vector.tensor_copy(ti32[:], t0[:])
                tf = t([128, M], f32, "tf")
                nc.vector.tensor_copy(tf[:], ti32[:])
                td = t([128, M], f32, "td")
                nc.vector.tensor_tensor(td[:], tf[:], t0[:], op=AL.is_gt)
                resf = t([128, M], f32, "resf")
                nc.vector.tensor_sub(resf[:], tf[:], td[:])
                locf = t([128, M], f32, "locf")
                nc.vector.tensor_tensor(locf[:], resf[:], iota[:], op=AL.max)
                loc16 = t([128, M], i16, "loc16")
                nc.vector.tensor_copy(loc16[:], locf[:])

                minn = t([128, 1], f32, "minn")
                nc.vector.tensor_scalar_min(minn[:], n_t[:], float(M))
                om = t([128, M], f32, "om")
                nc.vector.tensor_scalar(
                    om[:], iota[:], minn[:, 0:1], None, op0=AL.is_lt
                )

                if stage <= 2:
                    dbg = t([16, M], u8, "dbg")
                    nc.vector.tensor_copy(dbg[:], om[0:16, :])
                    nc.sync.dma_start(masko_d.ap(), dbg[:])
                    return

                # ---- phase 3: compaction scatters ----
                cmp_cidx = t([128, L], i16, "cmp_cidx")
                nc.gpsimd.local_scatter(
                    cmp_cidx[:], cidx16[:], sidx[:], 128, L, L
                )
                cmp_gpos = t([128, L], u16, "cmp_gpos")
                nc.gpsimd.local_scatter(
                    cmp_gpos[:], gpos16[:], sidx[:], 128, L, L
                )
                slotmap = t([128, L], i16, "slotmap")
                nc.gpsimd.local_scatter(
                    slotmap[:], data1[:], loc16[:], 128, L, M
                )
                idxs2 = t([128, L], i16, "idxs2")
                nc.vector.tensor_scalar_add(idxs2[:], slotmap[:], -1)
                featslot = t([128, M], i16, "featslot")
                nc.gpsimd.local_scatter(
                    featslot[:], cmp_cidx[:], idxs2[:], 128, M, L
                )
                gposslot = t([128, M], u16, "gposslot")
                nc.gpsimd.local_scatter(
                    gposslot[:], cmp_gpos[:], idxs2[:], 128, M, L
                )

                if stage <= 3:
                    dbg = t([16, M], u8, "dbg")
                    nc.vector.tensor_copy(dbg[:], featslot[0:16, :])
                    nc.sync.dma_start(masko_d.ap(), dbg[:])
                    return

                # ---- phase 4: finalize per-slot indices ----
                feat_f = t([128, M], f32, "feat_f")
                nc.vector.tensor_copy(feat_f[:], featslot[:])
                nc.vector.tensor_scalar_add(feat_f[:], feat_f[:], -float(NPTS))
                nc.vector.tensor_mul(feat_f[:], feat_f[:], om[:])
                nc.vector.tensor_scalar_add(feat_f[:], feat_f[:], float(NPTS))

                gpf = t([128, M], f32, "gpf")
                nc.vector.tensor_copy(gpf[:], gposslot[:])
                ph = t([128, M], f32, "ph")
                nc.vector.tensor_scalar_mul(ph[:], gpf[:], 0.5)
                pi32 = t([128, M], i32, "pi32")
                nc.vector.tensor_copy(pi32[:], ph[:])
                pf = t([128, M], f32, "pf")
                nc.vector.tensor_copy(pf[:], pi32[:])
                pd = t([128, M], f32, "pd")
                nc.vector.tensor_tensor(pd[:], pf[:], ph[:], op=AL.is_gt)
                pidx_f = t([128, M], f32, "pidx_f")
                nc.vector.tensor_sub(pidx_f[:], pf[:], pd[:])
                par_f = t([128, M], f32, "par_f")
                nc.vector.tensor_scalar_mul(par_f[:], pidx_f[:], -2.0)
                nc.vector.tensor_add(par_f[:], par_f[:], gpf[:])

                # ---- phase 5: extract this core's 16 rows via 0/1 matmuls ----
                pm_feat = psum.tile([128, M], f32, tag="pm_feat", name="pm_feat")
                nc.tensor.matmul(pm_feat[:], wt[:], feat_f[:], start=True, stop=True)
                pm_pidx = psum.tile([128, M], f32, tag="pm_pidx", name="pm_pidx")
                nc.tensor.matmul(pm_pidx[:], wt[:], pidx_f[:], start=True, stop=True)
                pm_par = psum.tile([128, M], f32, tag="pm_par", name="pm_par")
                nc.tensor.matmul(pm_par[:], wt[:], par_f[:], start=True, stop=True)
                pm_om = psum.tile([128, M], f32, tag="pm_om", name="pm_om")
                nc.tensor.matmul(pm_om[:16, :], w16t[:], om[:], start=True, stop=True)

                fidx_tile = t([128, IDXC], i16, "fidx_tile")
                nc.vector.memset(fidx_tile[:], -1)
                nc.vector.tensor_copy(fidx_tile[:, 0:M], pm_feat[:])
                pidx_tile = t([128, IDXC], i16, "pidx_tile")
                nc.vector.memset(pidx_tile[:], -1)
                nc.vector.tensor_copy(pidx_tile[:, 0:M], pm_pidx[:])

                mask16 = t([16, M], u8, "mask16")
                nc.vector.tensor_copy(mask16[:], pm_om[:16, :])
                nc.sync.dma_start(masko_d.ap(), mask16[:])

                if stage <= 4:
                    return

                # ---- phase 6: heavy gathers ----
                featsg = t([128, COLS * D], f32, "featsg")
                if stage != 6:
                    nc.gpsimd.dma_gather(
                        featsg[:].rearrange("p (c e) -> p c e", e=D),
                        feats_d.ap(),
                        fidx_tile[:],
                        NIDX,
                        NSLOT,
                        D,
                        single_packet=False,
                    )
                    featsg3 = featsg[:].rearrange("p (c e) -> p c e", e=D)
                    for g in range(8):
                        cg = (M - g + 7) // 8
                        nc.sync.dma_start(
                            featso_d.ap()[:, g:M:8, :],
                            featsg3[16 * g : 16 * (g + 1), 0:cg, :],
                        )
                if stage <= 5:
                    return

                ptsg = t([128, COLS * 64], f32, "ptsg")
                nc.gpsimd.dma_gather(
                    ptsg[:].rearrange("p (c e) -> p c e", e=64),
                    pts_d.ap(),
                    pidx_tile[:],
                    NIDX,
                    NSLOT,
                    64,
                    single_packet=False,
                )

                # ---- phase 7: point pairs back to proposal space, select ----
                ptsg3 = ptsg[:].rearrange("p (c e) -> p c e", e=64)
                pts6 = t([16, M * 6], f32, "pts6")
                pts6v = pts6[:].rearrange("p (i x) -> p i x", x=6)
                for g in range(8):
                    cg = (M - g + 7) // 8
                    nc.sync.dma_start(
                        pts6v[:, g:M:8, :],
                        ptsg3[16 * g : 16 * (g + 1), 0:cg, 0:6],
                    )
                par3 = t([16, M * 3], u8, "par3")
                par3v = par3[:].rearrange("p (i x) -> p i x", x=3)
                par16 = pm_par[:16, :].rearrange("p (i o) -> p i o", o=1)
                for x in range(3):
                    nc.vector.tensor_copy(par3v[:, :, x : x + 1], par16)
                podd = t([16, M * 3], f32, "podd")
                nc.vector.tensor_copy(
                    podd[:].rearrange("p (i x) -> p i x", x=3), pts6v[:, :, 3:6]
                )
                psel = t([16, M * 3], f32, "psel")
                nc.vector.tensor_copy(
                    psel[:].rearrange("p (i x) -> p i x", x=3), pts6v[:, :, 0:3]
                )
                nc.vector.copy_predicated(psel[:], par3[:], podd[:])
                nc.sync.dma_start(
                    ptso_d.ap(), psel[:].rearrange("p (i x) -> p i x", x=3)
                )

            body()

    nc.compile()
    return nc


def get_nc(stage=99):
    key = ("nc", stage)
    if key not in _CACHE:
        _CACHE[key] = _build_nc(stage)
    return _CACHE[key]


def prep_inputs(
    ref_node_neighbor_mask,
    ref_seed_neighbor_indices,
    ref_node_knn_masks,
    ref_node_knn_points,
    ref_node_knn_indices,
    ref_feats_m,
):
    """Shard/marshal inputs -> list of 8 per-core input maps."""
    nb_mask = np.asarray(ref_node_neighbor_mask).astype(bool)
    seed = np.asarray(ref_seed_neighbor_indices).astype(np.int64)
    knn_mask = np.asarray(ref_node_knn_masks).astype(bool)
    knn_pts = np.asarray(ref_node_knn_points).astype(np.float32)
    knn_idx = np.asarray(ref_node_knn_indices).astype(np.int64)
    feats = np.asarray(ref_feats_m).astype(np.float32)

    seed_eff = np.where(nb_mask, seed, NNODES).astype(np.int64)

    s = np.arange(P * NB)
    val = seed_eff[s % 128, s // 128].astype(np.int16)
    seedw = np.tile(val.reshape(128, 16).T, (8, 1)).astype(np.int16)
    seedw = np.ascontiguousarray(seedw)

    mask_tbl = np.zeros((NNODES + 1, K), np.float32)
    mask_tbl[:NNODES] = knn_mask
    cidx_tbl = np.zeros((NNODES + 1, K), np.float32)
    cidx_tbl[:NNODES] = knn_idx
    gpos2_tbl = (
        np.arange(NNODES + 1, dtype=np.float32)[:, None] * K
        + np.arange(K, dtype=np.float32)[None, :]
        + 2.0
    ).astype(np.float32)

    flat = knn_pts.reshape(NNODES * K, 3)
    ptspair = np.zeros((NPAIR + 1, 64), np.float32)
    ptspair[1:, 0:3] = flat[0::2]
    ptspair[1:, 3:6] = flat[1::2]

    feats_pad = np.zeros((NPTS + 1, D), np.float32)
    feats_pad[:NPTS] = feats

    frac_bc = np.tile(
        np.arange(M, dtype=np.float32) / np.float32(M), (128, 1)
    ).astype(np.float32)
    iota_bc = np.tile(np.arange(M, dtype=np.float32), (128, 1)).astype(np.float32)
    data1 = np.tile(np.arange(1, M + 1, dtype=np.int16), (128, 1)).astype(np.int16)

    m = np.arange(128)
    in_maps = []
    for c in range(NCORES):
        W = np.zeros((128, 128), np.float32)
        W[16 * c + (m % 16), m] = 1.0
        W16 = np.zeros((128, 16), np.float32)
        W16[16 * c + np.arange(16), np.arange(16)] = 1.0
        in_maps.append(
            {
                "seedw": seedw,
                "mask_tbl": mask_tbl,
                "cidx_tbl": cidx_tbl,
                "gpos2_tbl": gpos2_tbl,
                "ptspair": ptspair,
                "feats_pad": feats_pad,
                "frac_bc": frac_bc,
                "iota_bc": iota_bc,
                "data1": data1,
                "W": W,
                "W16": W16,
            }
        )
    return in_maps


def assemble_outputs(results):
    """results: list of 8 per-core output dicts -> full (pts, feats, mask)."""
    feats = np.concatenate([r["feats_out"] for r in results], axis=0)
    pts = np.concatenate([r["pts_out"] for r in results], axis=0)
    mask = np.concatenate([r["mask_out"] for r in results], axis=0).astype(bool)
    return pts, feats, mask


def kernel(**inputs):
    from concourse.bass_utils import run_bass_kernel_spmd

    nc = get_nc()
    in_maps = prep_inputs(**inputs)
    res = run_bass_kernel_spmd(nc, in_maps, core_ids=list(range(NCORES)))
    return assemble_outputs(res.results)


# revision 15
# speedup vs baseline: 1.1483x; 1.1483x over previous
"""Trainium2 Bass kernel for nn_Node2Point (ragged gather/compact/resample).

Strategy: data-parallel over the 128 proposals across 8 cores (16 each).
Every core runs the identical cheap index computation for all 128 proposals
with partition-dim = proposal; per-core 0/1 matmuls extract that core's
16-proposal slice into the wrapped index-tile layout that dma_gather wants;
each core then performs only its own heavy feature-row gather and writes its
output shard.
"""

import sys

if "/opt/trn_rl_repo" not in sys.path:
    sys.path.insert(0, "/opt/trn_rl_repo")

import numpy as np

# dims (hardcoded per problem spec)
P, NB, K, NNODES, NPTS, D = 128, 16, 64, 512, 20000, 256
M = 450                 # FINEMATCH_MAX_POINT
L = NB * K              # 1024
NCORES = 8
PLOC = P // NCORES      # 16 proposals per core
NSLOT = PLOC * M        # 7200 gather slots per core
NIDX = 7296             # rounded up to multiple of 128 (456 idx cols * 16)
COLS = NIDX // 128      # 57
IDXC = NIDX // 16       # 456
NPAIR = NNODES * K // 2  # 16384 point pairs

_CACHE = {}


def _build_nc(stage=99):
    import concourse.mybir as mybir
    from concourse import bacc, tile

    AL = mybir.AluOpType
    f32, i16, u16, i32, u8 = (
        mybir.dt.float32,
        mybir.dt.int16,
        mybir.dt.uint16,
        mybir.dt.int32,
        mybir.dt.uint8,
    )

    nc = bacc.Bacc("TRN2", debug=False, num_devices=NCORES)

    # inputs
    seedw_d = nc.dram_tensor("seedw", [128, 128], i16, kind="ExternalInput")
    fused_d = nc.dram_tensor("fused_tbl", [NNODES + 1, 3 * K], f32, kind="ExternalInput")
    pts_d = nc.dram_tensor("ptspair", [NPAIR + 1, 64], f32, kind="ExternalInput")
    feats_d = nc.dram_tensor("feats_pad", [NPTS + 1, D], f32, kind="ExternalInput")
    frac_d = nc.dram_tensor("frac_bc", [128, M], f32, kind="ExternalInput")
    iota_d = nc.dram_tensor("iota_bc", [128, M], f32, kind="ExternalInput")
    data1_d = nc.dram_tensor("data1", [128, M], i16, kind="ExternalInput")
    w_d = nc.dram_tensor("W", [128, 128], f32, kind="ExternalInput")
    w16_d = nc.dram_tensor("W16", [128, 16], f32, kind="ExternalInput")

    # outputs (per-core shard)
    featso_d = nc.dram_tensor("feats_out", [PLOC, M, D], f32, kind="ExternalOutput")
    ptso_d = nc.dram_tensor("pts_out", [PLOC, M, 3], f32, kind="ExternalOutput")
    masko_d = nc.dram_tensor("mask_out", [PLOC, M], u8, kind="ExternalOutput")

    with tile.TileContext(nc) as tc:
        with (
            tc.tile_pool(name="main", bufs=1) as pool,
            tc.tile_pool(name="ps", bufs=1, space="PSUM") as psum,
        ):
            t = lambda shape, dt, tag: pool.tile(shape, dt, tag=tag, name=tag)

            def body():
                # ---- load small inputs ----
                seedw = t([128, 128], i16, "seedw")
                nc.sync.dma_start(seedw[:], seedw_d.ap())
                if stage <= 0:
                    dbg0 = t([16, M], u8, "dbg0")
                    nc.vector.tensor_copy(dbg0[:], seedw[0:16, 0:128].broadcast_to([16, M]) if False else seedw[0:16, 0:1].broadcast_to([16, M]))
                    nc.sync.dma_start(masko_d.ap(), dbg0[:])
                    return
                frac = t([128, M], f32, "frac")
                nc.sync.dma_start(frac[:], frac_d.ap())
                iota = t([128, M], f32, "iota")
                nc.sync.dma_start(iota[:], iota_d.ap())
                data1 = t([128, M], i16, "data1")
                nc.sync.dma_start(data1[:], data1_d.ap())
                wt = t([128, 128], f32, "wt")
                nc.sync.dma_start(wt[:], w_d.ap())
                w16t = t([128, 16], f32, "w16t")
                nc.sync.dma_start(w16t[:], w16_d.ap())

                # ---- phase 1: fused table gather -> [128, 16, 192] ----
                g_all = t([128, NB * 3 * K], f32, "g_all")
                g_all3 = g_all[:].rearrange("p (c k) -> p c k", k=3 * K)
                nc.gpsimd.dma_gather(
                    g_all3,
                    fused_d.ap(),
                    seedw[:],
                    P * NB,
                    P * NB,
                    3 * K,
                    single_packet=False,
                )
                g_mask = t([128, L], f32, "g_mask")
                nc.vector.tensor_copy(
                    g_mask[:].rearrange("p (c k) -> p c k", k=K),
                    g_all3[:, :, 0:K],
                )

                if stage <= 1:
                    dbg = t([16, M], u8, "dbg")
                    nc.vector.tensor_copy(dbg[:], g_mask[0:16, 0:M])
                    nc.sync.dma_start(masko_d.ap(), dbg[:])
                    return

                # ---- phase 2: scan / ranks / resample ----
                zeros = t([128, L], f32, "zeros")
                nc.vector.memset(zeros[:], 0.0)
                incl = t([128, L], f32, "incl")
                nc.vector.tensor_tensor_scan(
                    incl[:], g_mask[:], zeros[:], 0.0, AL.add, AL.add
                )
                n_t = t([128, 1], f32, "n_t")
                nc.vector.tensor_reduce(
                    n_t[:], g_mask[:], axis=mybir.AxisListType.X, op=AL.add
                )

                sidx_f = t([128, L], f32, "sidx_f")
                nc.vector.tensor_mul(sidx_f[:], incl[:], g_mask[:])
                sidx = t([128, L], i16, "sidx")
                nc.vector.tensor_scalar_add(sidx[:], sidx_f[:], -1.0)

                cidx16 = t([128, L], i16, "cidx16")
                nc.vector.tensor_copy(
                    cidx16[:].rearrange("p (c k) -> p c k", k=K),
                    g_all3[:, :, K : 2 * K],
                )
                gpos16 = t([128, L], u16, "gpos16")
                nc.vector.tensor_copy(
                    gpos16[:].rearrange("p (c k) -> p c k", k=K),
                    g_all3[:, :, 2 * K : 3 * K],
                )

                # local[i] = max(floor(frac*n), i)
                t0 = t([128, M], f32, "t0")
                nc.vector.tensor_scalar(
                    t0[:], frac[:], n_t[:, 0:1], None, op0=AL.mult
                )
                ti32 = t([128, M], i32, "ti32")
                nc.vector.tensor_copy(ti32[:], t0[:])
                tf = t([128, M], f32, "tf")
                nc.vector.tensor_copy(tf[:], ti32[:])
                td = t([128, M], f32, "td")
                nc.vector.tensor_tensor(td[:], tf[:], t0[:], op=AL.is_gt)
                resf = t([128, M], f32, "resf")
                nc.vector.tensor_sub(resf[:], tf[:], td[:])
                loc16 = t([128, M], i16, "loc16")
                nc.vector.tensor_tensor(loc16[:], resf[:], iota[:], op=AL.max)

                minn = t([128, 1], f32, "minn")
                nc.vector.tensor_scalar_min(minn[:], n_t[:], float(M))
                om = t([128, M], f32, "om")
                nc.vector.tensor_scalar(
                    om[:], iota[:], minn[:, 0:1], None, op0=AL.is_lt
                )

                if stage <= 2:
                    dbg = t([16, M], u8, "dbg")
                    nc.vector.tensor_copy(dbg[:], om[0:16, :])
                    nc.sync.dma_start(masko_d.ap(), dbg[:])
                    return

                # ---- phase 3: compaction scatters ----
                cmp_cidx = t([128, L], i16, "cmp_cidx")
                nc.gpsimd.local_scatter(
                    cmp_cidx[:], cidx16[:], sidx[:], 128, L, L
                )
                cmp_gpos = t([128, L], u16, "cmp_gpos")
                nc.gpsimd.local_scatter(
                    cmp_gpos[:], gpos16[:], sidx[:], 128, L, L
                )
                slotmap = t([128, L], i16, "slotmap")
                nc.gpsimd.local_scatter(
                    slotmap[:], data1[:], loc16[:], 128, L, M
                )
                idxs2 = t([128, L], i16, "idxs2")
                nc.vector.tensor_scalar_add(idxs2[:], slotmap[:], -1)
                featslot = t([128, M], i16, "featslot")
                nc.gpsimd.local_scatter(
                    featslot[:], cmp_cidx[:], idxs2[:], 128, M, L
                )
                gposslot = t([128, M], u16, "gposslot")
                nc.gpsimd.local_scatter(
                    gposslot[:], cmp_gpos[:], idxs2[:], 128, M, L
                )

                if stage <= 3:
                    dbg = t([16, M], u8, "dbg")
                    nc.vector.tensor_copy(dbg[:], featslot[0:16, :])
                    nc.sync.dma_start(masko_d.ap(), dbg[:])
                    return

                # ---- phase 4: finalize per-slot indices ----
                feat_f = t([128, M], f32, "feat_f")
                nc.vector.tensor_copy(feat_f[:], featslot[:])
                nc.vector.tensor_scalar_add(feat_f[:], feat_f[:], -float(NPTS))
                nc.vector.tensor_mul(feat_f[:], feat_f[:], om[:])
                nc.vector.tensor_scalar_add(feat_f[:], feat_f[:], float(NPTS))

                gpf = t([128, M], f32, "gpf")
                nc.vector.tensor_copy(gpf[:], gposslot[:])
                ph = t([128, M], f32, "ph")
                nc.vector.tensor_scalar_mul(ph[:], gpf[:], 0.5)
                pi32 = t([128, M], i32, "pi32")
                nc.vector.tensor_copy(pi32[:], ph[:])
                pf = t([128, M], f32, "pf")
                nc.vector.tensor_copy(pf[:], pi32[:])
                pd = t([128, M], f32, "pd")
                nc.vector.tensor_tensor(pd[:], pf[:], ph[:], op=AL.is_gt)
                pidx_f = t([128, M], f32, "pidx_f")
                nc.vector.tensor_sub(pidx_f[:], pf[:], pd[:])
                par_f = t([128, M], f32, "par_f")
                nc.vector.tensor_scalar_mul(par_f[:], pidx_f[:], -2.0)
                nc.vector.tensor_add(par_f[:], par_f[:], gpf[:])

                # ---- phase 5: extract this core's 16 rows via 0/1 matmuls ----
                pm_feat = psum.tile([128, M], f32, tag="pm_feat", name="pm_feat")
                nc.tensor.matmul(pm_feat[:], wt[:], feat_f[:], start=True, stop=True)
                pm_pidx = psum.tile([128, M], f32, tag="pm_pidx", name="pm_pidx")
                nc.tensor.matmul(pm_pidx[:], wt[:], pidx_f[:], start=True, stop=True)
                pm_par = psum.tile([128, M], f32, tag="pm_par", name="pm_par")
                nc.tensor.matmul(pm_par[:], wt[:], par_f[:], start=True, stop=True)
                pm_om = psum.tile([128, M], f32, tag="pm_om", name="pm_om")
                nc.tensor.matmul(pm_om[:16, :], w16t[:], om[:], start=True, stop=True)

                fidx_tile = t([128, IDXC], i16, "fidx_tile")
                nc.vector.memset(fidx_tile[:], -1)
                nc.vector.tensor_copy(fidx_tile[:, 0:M], pm_feat[:])
                pidx_tile = t([128, IDXC], i16, "pidx_tile")
                nc.vector.memset(pidx_tile[:], -1)
                nc.vector.tensor_copy(pidx_tile[:, 0:M], pm_pidx[:])

                mask16 = t([16, M], u8, "mask16")
                nc.vector.tensor_copy(mask16[:], pm_om[:16, :])
                nc.sync.dma_start(masko_d.ap(), mask16[:])

                if stage <= 4:
                    return

                # ---- phase 6: heavy gathers (pts first; feats chunked) ----
                ptsg = t([128, COLS * 64], f32, "ptsg")
                nc.gpsimd.dma_gather(
                    ptsg[:].rearrange("p (c e) -> p c e", e=64),
                    pts_d.ap(),
                    pidx_tile[:],
                    NIDX,
                    NSLOT,
                    64,
                    single_packet=False,
                )

                featsg = t([128, COLS * D], f32, "featsg")
                featsg3 = featsg[:].rearrange("p (c e) -> p c e", e=D)
                # chunks in units of sbuf cols (128 slots each): 15/15/15/12
                chunk_cols = [15, 15, 15, 12]
                c0 = 0
                for ck, ccols in enumerate(chunk_cols):
                    slots = ccols * 128
                    s0 = c0 * 128
                    valid = max(0, min(NSLOT - s0, slots))
                    nc.gpsimd.dma_gather(
                        featsg3[:, c0 : c0 + ccols, :],
                        feats_d.ap(),
                        fidx_tile[:, c0 * 8 : (c0 + ccols) * 8],
                        slots,
                        valid,
                        D,
                        single_packet=False,
                    )
                    # writeback this chunk: i-range [c0*8, (c0+ccols)*8)
                    i0, i1 = c0 * 8, min((c0 + ccols) * 8, M)
                    for g in range(8):
                        gi0 = i0 + ((g - i0) % 8)
                        if gi0 >= i1:
                            continue
                        ncols = (i1 - gi0 + 7) // 8
                        nc.sync.dma_start(
                            featso_d.ap()[:, gi0:i1:8, :],
                            featsg3[
                                16 * g : 16 * (g + 1),
                                c0 + (gi0 - i0) // 8 : c0 + (gi0 - i0) // 8 + ncols,
                                :,
                            ],
                        )
                    c0 += ccols

                # ---- phase 7: point pairs back to proposal space, select ----
                ptsg3 = ptsg[:].rearrange("p (c e) -> p c e", e=64)
                pts6 = t([16, M * 6], f32, "pts6")
                pts6v = pts6[:].rearrange("p (i x) -> p i x", x=6)
                for g in range(8):
                    cg = (M - g + 7) // 8
                    nc.sync.dma_start(
                        pts6v[:, g:M:8, :],
                        ptsg3[16 * g : 16 * (g + 1), 0:cg, 0:6],
                    )
                par3 = t([16, M * 3], u8, "par3")
                par3v = par3[:].rearrange("p (i x) -> p i x", x=3)
                par16 = pm_par[:16, :].rearrange("p (i o) -> p i o", o=1)
                for x in range(3):
                    nc.vector.tensor_copy(par3v[:, :, x : x + 1], par16)
                podd = t([16, M * 3], f32, "podd")
                nc.vector.tensor_copy(
                    podd[:].rearrange("p (i x) -> p i x", x=3), pts6v[:, :, 3:6]
                )
                psel = t([16, M * 3], f32, "psel")
                nc.vector.tensor_copy(
                    psel[:].rearrange("p (i x) -> p i x", x=3), pts6v[:, :, 0:3]
                )
                nc.vector.copy_predicated(psel[:], par3[:], podd[:])
                nc.sync.dma_start(
                    ptso_d.ap(), psel[:].rearrange("p (i x) -> p i x", x=3)
                )

            body()

    nc.compile()
    return nc


def get_nc(stage=99):
    key = ("nc", stage)
    if key not in _CACHE:
        _CACHE[key] = _build_nc(stage)
    return _CACHE[key]


def prep_inputs(
    ref_node_neighbor_mask,
    ref_seed_neighbor_indices,
    ref_node_knn_masks,
    ref_node_knn_points,
    ref_node_knn_indices,
    ref_feats_m,
):
    """Shard/marshal inputs -> list of 8 per-core input maps."""
    nb_mask = np.asarray(ref_node_neighbor_mask).astype(bool)
    seed = np.asarray(ref_seed_neighbor_indices).astype(np.int64)
    knn_mask = np.asarray(ref_node_knn_masks).astype(bool)
    knn_pts = np.asarray(ref_node_knn_points).astype(np.float32)
    knn_idx = np.asarray(ref_node_knn_indices).astype(np.int64)
    feats = np.asarray(ref_feats_m).astype(np.float32)

    seed_eff = np.where(nb_mask, seed, NNODES).astype(np.int64)

    s = np.arange(P * NB)
    val = seed_eff[s % 128, s // 128].astype(np.int16)
    seedw = np.tile(val.reshape(128, 16).T, (8, 1)).astype(np.int16)
    seedw = np.ascontiguousarray(seedw)

    fused_tbl = np.zeros((NNODES + 1, 3 * K), np.float32)
    fused_tbl[:NNODES, 0:K] = knn_mask
    fused_tbl[:NNODES, K : 2 * K] = knn_idx
    fused_tbl[:, 2 * K : 3 * K] = (
        np.arange(NNODES + 1, dtype=np.float32)[:, None] * K
        + np.arange(K, dtype=np.float32)[None, :]
        + 2.0
    )

    flat = knn_pts.reshape(NNODES * K, 3)
    ptspair = np.zeros((NPAIR + 1, 64), np.float32)
    ptspair[1:, 0:3] = flat[0::2]
    ptspair[1:, 3:6] = flat[1::2]

    feats_pad = np.zeros((NPTS + 1, D), np.float32)
    feats_pad[:NPTS] = feats

    frac_bc = np.tile(
        np.arange(M, dtype=np.float32) / np.float32(M), (128, 1)
    ).astype(np.float32)
    iota_bc = np.tile(np.arange(M, dtype=np.float32), (128, 1)).astype(np.float32)
    data1 = np.tile(np.arange(1, M + 1, dtype=np.int16), (128, 1)).astype(np.int16)

    m = np.arange(128)
    in_maps = []
    for c in range(NCORES):
        W = np.zeros((128, 128), np.float32)
        W[16 * c + (m % 16), m] = 1.0
        W16 = np.zeros((128, 16), np.float32)
        W16[16 * c + np.arange(16), np.arange(16)] = 1.0
        in_maps.append(
            {
                "seedw": seedw,
                "fused_tbl": fused_tbl,
                "ptspair": ptspair,
                "feats_pad": feats_pad,
                "frac_bc": frac_bc,
                "iota_bc": iota_bc,
                "data1": data1,
                "W": W,
                "W16": W16,
            }
        )
    return in_maps


def assemble_outputs(results):
    """results: list of 8 per-core output dicts -> full (pts, feats, mask)."""
    feats = np.concatenate([r["feats_out"] for r in results], axis=0)
    pts = np.concatenate([r["pts_out"] for r in results], axis=0)
    mask = np.concatenate([r["mask_out"] for r in results], axis=0).astype(bool)
    return pts, feats, mask


def kernel(**inputs):
    from concourse.bass_utils import run_bass_kernel_spmd

    nc = get_nc()
    in_maps = prep_inputs(**inputs)
    res = run_bass_kernel_spmd(nc, in_maps, core_ids=list(range(NCORES)))
    return assemble_outputs(res.results)
